# revision 9
# baseline (speedup 1.0000x reference)
"""Trainium2 Bass kernel for CosineAttention.

Model (fp32 reference):
  q = l2norm_head(x @ Wq.T + bq), k = l2norm_head(x @ Wk.T + bk), v = x @ Wv.T + bv
  attn = softmax(tau_h * (q . k) + mask), out = (attn @ v) @ Wo.T + bo

Sharding: B*H = 2*16 = 32 (batch, head) units over 8 cores -> each core owns
one batch (b = core//4) and 4 heads. q/k/v projections are column-sharded
(rows of W), the output projection row-sharded; partial [DM, T] outputs are
summed per batch on the host (the unshard step). tau shards with heads.

All heavy matmuls run with bf16 operands accumulating in fp32 PSUM. All
layout transposes AND relayouts are done on the HOST (free): every DMA the
kernel issues is a plain contiguous [128, N] block, so descriptor
generation is cheap (~0.6us issue each, ~10 DMAs total). Scores are
computed transposed: S^T[k, q] = k-hat @ q-hat^T so softmax needs no max
subtraction (|tau*cos| <= tau) and exp(S^T) feeds the AV matmul directly as
the moving operand. An appended ones-column on V gives the softmax
denominator for free in PSUM row 64. Causal masks are lowered to skipping
fully masked S^T blocks plus 0/1 staircase multiplies on diagonal blocks.

Perf structure (v2):
- A 16-matmul dummy accumulation group + dummy Ln activation run during the
  DMA wait: they warm the PE HAM clock gate (1.2 -> 2.4 GHz) and load the
  ACT table set off the critical path.
- Attention processes heads in PAIRS with a 2-deep software pipeline:
  iteration i issues score(i) [adjacent K=64 matmuls on disjoint array row
  groups -> they run concurrently], exp(i-1) on Scalar, and AV(i-2) on the
  PE. The AV's exp finished a full iteration earlier, so at most one PE
  instruction per iteration blocks on a semaphore.
- PSUM: scores cycle 4 banks, u accumulators 2, projection/output 2; norm
  sums and the warmup share the projection pool, the denominator broadcast
  borrows a score bank at pair tails.
- Softmax denominator: both heads' sums sit in a [2,512] tile, one
  single-op DVE approx reciprocal (~18 bits), then ONE block-diagonal K=2
  matmul broadcasts both rows and ONE full-width multiply normalizes the
  pair.
- q/k-norm Square runs on Scalar straight from PSUM while the bias-free qb
  copy runs on DVE, balancing the two engines.
"""

import numpy as np
from contextlib import ExitStack

import ml_dtypes

import concourse.bass as bass
import concourse.mybir as mybir
import concourse.tile as tile
from concourse import bacc
from concourse.bass_utils import run_bass_kernel_spmd

B, T, DM, H = 2, 2048, 1024, 16
D = 64
NCORES = 8
HPC = 4            # heads per core
HD = HPC * D       # 256 head dims per core
NT = T // 128      # 16 k-blocks / T-tiles
NQC = T // 512     # 4 q-chunks
NKD = DM // 128    # 8 contraction chunks of DM
F32 = mybir.dt.float32
F32R = mybir.dt.float32r
BF16 = mybir.dt.bfloat16
AF = mybir.ActivationFunctionType
BF = ml_dtypes.bfloat16


def build_program(variant: str, bz: bool) -> bass.Bass:
    """variant: 'causal' | 'zeros' | 'mask'; bz: all biases are zero"""
    assert variant in ("causal", "zeros", "mask")
    nc = bacc.Bacc("TRN2", target_bir_lowering=False, debug=False,
                   num_devices=NCORES)

    # all big tensors pre-permuted on host so DMAs are contiguous blocks
    xt_p = nc.declare_dram_parameter("xt", [NQC * 128, NKD * 512], BF16,
                                     isOutput=False)
    wqkt_p = nc.declare_dram_parameter("wqkt", [128, NKD * 512], BF16,
                                       isOutput=False)
    wvt_p = nc.declare_dram_parameter("wvt", [128, NKD * HD], BF16,
                                      isOutput=False)
    wot_p = nc.declare_dram_parameter("wot", [128, 2 * DM], BF16,
                                      isOutput=False)
    bqk_p = nc.declare_dram_parameter("bqk", [128, 4], F32, isOutput=False)
    bvr_p = nc.declare_dram_parameter("bvr", [1, HD], BF16, isOutput=False)
    bo_p = nc.declare_dram_parameter("bo", [128, NKD], F32, isOutput=False)
    tblk_p = nc.declare_dram_parameter("tblk", [8, 512], F32, isOutput=False)
    oblk_p = nc.declare_dram_parameter("oblk", [128, 4, 8], BF16, isOutput=False)
    if variant == "causal":
        pat_p = nc.declare_dram_parameter("pat", [128, 4, 512], BF16, isOutput=False)
    if variant == "mask":
        mt_p = nc.declare_dram_parameter("maskt", [T, T], F32, isOutput=False)
    yt_p = nc.declare_dram_parameter("yt", [NQC * 128, NKD * 512], BF16,
                                     isOutput=True)

    with tile.TileContext(nc) as tc, ExitStack() as top:
        const = top.enter_context(tc.tile_pool(name="const", bufs=1))
        wts = top.enter_context(tc.tile_pool(name="wts", bufs=1))
        acts = top.enter_context(tc.tile_pool(name="acts", bufs=1))

        # ---- weights / activations resident in SBUF ----
        wqkt = wts.tile([128, NKD, 2 * HD], BF16, tag="wqkt")
        wvt = wts.tile([128, NKD, HD], BF16, tag="wvt")
        wot = wts.tile([128, 2, DM], BF16, tag="wot")
        # chunk-major so each chunk's DMA is one contiguous block
        xts = wts.tile([128, NQC, NKD, 512], BF16, tag="xts")

        qhatT = acts.tile([128, 2, T], BF16, tag="qhatT")
        khatT = acts.tile([128, 2, T], BF16, tag="khatT")
        v_ext = acts.tile([128, NT, HPC, D + 1], BF16, tag="v_ext")
        aT = acts.tile([128, 2, T], BF16, tag="aT")

        # ---- DMA: one contiguous transfer per tensor / chunk ----
        nc.sync.dma_start(wqkt[:],
                          wqkt_p.ap().rearrange("p (a f) -> p a f", f=512))
        nc.sync.dma_start(xts[:, 0, :, :],
                          xt_p.ap()[0:128, :].rearrange("p (a f) -> p a f",
                                                        f=512))
        nc.sync.dma_start(wvt[:],
                          wvt_p.ap().rearrange("p (a f) -> p a f", f=HD))
        if not bz:
            bqk = const.tile([128, 4], F32)
            nc.sync.dma_start(bqk[:], bqk_p.ap())
            bvr = const.tile([1, HD], BF16)
            nc.sync.dma_start(bvr[:], bvr_p.ap())
        tblk_f = const.tile([8, 512], F32)
        nc.sync.dma_start(tblk_f[:], tblk_p.ap())
        oblk = const.tile([128, 4, 8], BF16)
        nc.sync.dma_start(oblk[:], oblk_p.ap())
        if variant == "causal":
            pat_sb = const.tile([128, 4, 512], BF16)
            nc.sync.dma_start(pat_sb[:], pat_p.ap())
        for tc_i in range(1, NQC):
            nc.sync.dma_start(
                xts[:, tc_i, :, :],
                xt_p.ap()[tc_i * 128:(tc_i + 1) * 128, :]
                .rearrange("p (a f) -> p a f", f=512))
        nc.sync.dma_start(wot[:],
                          wot_p.ap().rearrange("p (a f) -> p a f", f=DM))
        if not bz:
            bo_sb = const.tile([128, NKD], F32)
            nc.sync.dma_start(bo_sb[:], bo_p.ap())

        # ---- small constants ----
        ones_f = const.tile([1, 128], F32)
        nc.vector.memset(ones_f[:], 1.0)
        ones_b = const.tile([1, 128], BF16)
        nc.vector.tensor_copy(ones_b[:], ones_f[:])
        ones64_r = const.tile([1, 64], F32R)
        nc.vector.tensor_copy(ones64_r[:], ones_f[:, 0:64])
        tblk_r = const.tile([8, 512], F32R)
        nc.vector.tensor_copy(tblk_r[:], tblk_f[:])
        onesv_f = const.tile([128, NT * HPC], F32)
        nc.vector.memset(onesv_f[:], 1.0)
        # ones column of v_ext (softmax denominator trick)
        nc.vector.tensor_copy(
            v_ext[:, :, :, D:D + 1],
            onesv_f[:].rearrange("p (a b c) -> p a b c", a=NT, b=HPC))
        # warm-up operands
        wrm = const.tile([128, 512], BF16)
        nc.vector.memset(wrm[:], 0.001)
        lnw = const.tile([1, 8], F32)

        # ---- pools ----
        uapool = top.enter_context(tc.tile_pool(name="uapool", bufs=2))
        gpool = top.enter_context(tc.tile_pool(name="gpool", bufs=3))
        qkpool = top.enter_context(tc.tile_pool(name="qkpool", bufs=2))
        sqpool = top.enter_context(tc.tile_pool(name="sqpool", bufs=2))
        espool = top.enter_context(tc.tile_pool(name="espool", bufs=8))
        ypool = top.enter_context(tc.tile_pool(name="ypool", bufs=2))
        if variant == "mask":
            mpool = top.enter_context(tc.tile_pool(name="mpool", bufs=NT))
        # PSUM budget (8 banks): scores/bcast 4 + u 2 + proj/y/norm 2
        spsum = top.enter_context(
            tc.tile_pool(name="spsum", bufs=4, space="PSUM"))
        upsum = top.enter_context(
            tc.tile_pool(name="upsum", bufs=2, space="PSUM"))
        proj_psum = top.enter_context(
            tc.tile_pool(name="proj_psum", bufs=2, space="PSUM"))

        # ---- warm-up: PE HAM + ACT table load, during the DMA wait ----
        nc.scalar.activation(lnw[:], ones_f[:, 0:8], AF.Ln)
        wps = proj_psum.tile([128, 512], F32, tag="proj", name="warm")
        for i in range(16):
            nc.tensor.matmul(wps[:], wrm[:, 0:128], wrm[:],
                             start=(i == 0), stop=(i == 15))

        def proj_chunk(tcx, after_first=None):
            """project tokens [tcx*512, (tcx+1)*512) -> qhatT/khatT/v_ext.

            after_first, if given, is called after the first matmul group is
            emitted — used to flush the previous attention chunk's trailing
            exp/AV so its exp lands early in the scalar queue while the PE
            already has projection work."""
            ts = slice(tcx * 512, (tcx + 1) * 512)
            qb = qkpool.tile([128, 4, 512], F32, tag="qb", name=f"qb{tcx}")
            sq = sqpool.tile([128, 4, 512], BF16, tag="sq", name=f"sq{tcx}")

            def v_tile(tt):
                t = tcx * 4 + tt
                pv = proj_psum.tile([128, HD], F32, tag="proj",
                                    name=f"pv{t}")
                for dk in range(NKD):
                    nc.tensor.matmul(pv[:],
                                     xts[:, tcx, dk, tt * 128:(tt + 1) * 128],
                                     wvt[:, dk, :],
                                     start=(dk == 0), stop=(bz and dk == NKD - 1))
                if not bz:
                    nc.tensor.matmul(pv[:], ones_b[:], bvr[:],
                                     start=False, stop=True)
                nc.vector.tensor_copy(v_ext[:, t, :, 0:D],
                                      pv[:].rearrange("p (h d) -> p h d", h=HPC))

            for j in range(4):
                pj = proj_psum.tile([128, 512], F32, tag="proj",
                                    name=f"pj{tcx}_{j}")
                for dk in range(NKD):
                    nc.tensor.matmul(pj[:],
                                     wqkt[:, dk, j * 128:(j + 1) * 128],
                                     xts[:, tcx, dk, :],
                                     start=(dk == 0), stop=(dk == NKD - 1))
                if j == 0 and after_first is not None:
                    after_first()
                if tcx == 0 and j >= 2:
                    # chunk 0 has no attention stream to hide latency; V
                    # tiles are the only independent PE work
                    v_tile(j - 2)
                if bz:
                    # Square on Scalar straight from PSUM; plain copy on DVE
                    nc.vector.tensor_copy(qb[:, j, :], pj[:])
                    nc.scalar.activation(sq[:, j, :], pj[:], AF.Square)
                else:
                    nc.scalar.activation(qb[:, j, :], pj[:], AF.Identity,
                                         bias=bqk[:, j:j + 1])
                    nc.vector.tensor_mul(sq[:, j, :], qb[:, j, :], qb[:, j, :])

            if tcx != 0:
                v_tile(0)
                v_tile(1)
            else:
                v_tile(2)
            ns = proj_psum.tile([8, 512], F32, tag="proj", name=f"ns{tcx}")
            for j in range(4):
                nc.tensor.matmul(ns[:], oblk[:, j, :], sq[:, j, :],
                                 start=(j == 0), stop=(j == 3))
            # rsqrt = exp(-0.5*ln): short chain, both on Scalar; the Exp
            # writes f32r directly so no cast sits before the broadcasts
            sn = gpool.tile([8, 512], F32, tag="g", name=f"sn{tcx}")
            nc.scalar.activation(sn[:], ns[:], AF.Ln)
            rr = gpool.tile([8, 512], F32R, tag="g", name=f"rr{tcx}")
            nc.scalar.activation(rr[:], sn[:], AF.Exp, scale=-0.5)

            def bcast(j):
                bcp = spsum.tile([128, 512], F32, tag="s",
                                 name=f"bcp{tcx}_{j}")
                nc.tensor.matmul(bcp[:], tblk_r[:, j * 128:(j + 1) * 128],
                                 rr[:], start=True, stop=True)
                dst = qhatT if j < 2 else khatT
                nc.vector.tensor_mul(dst[:, j % 2, ts], qb[:, j, :], bcp[:])

            if tcx != 0:
                v_tile(2)
                bcast(0)
                bcast(1)
                v_tile(3)
            else:
                v_tile(3)
                bcast(0)
                bcast(1)
            bcast(2)
            bcast(3)

        proj_chunk(0)
        oproj_pend = None

        for qc in range(NQC):
            kbs = list(range(4 * qc + 4)) if variant == "causal" else list(range(NT))
            nkb = len(kbs)
            mks = {}
            if variant == "mask":
                for kb in kbs:
                    mk = mpool.tile([128, 512], F32, tag="mk",
                                    name=f"mk{qc}_{kb}")
                    nc.sync.dma_start(
                        mk[:], mt_p.ap()[kb * 128:(kb + 1) * 128,
                                         qc * 512:(qc + 1) * 512])
                    mks[kb] = mk

            def blk_off(kb):
                """first possibly-unmasked column of this S^T block"""
                if variant != "causal":
                    return 0
                ai = kb - 4 * qc
                return ai * 128 if ai > 0 else 0

            def score_mm(h, kb):
                hp, ho = h // 2, (h % 2) * 64
                off = blk_off(kb)
                sp = spsum.tile([128, 512], F32, tag="s",
                                name=f"s{qc}_{kb}_{h}")
                nc.tensor.matmul(
                    sp[:, off:512],
                    khatT[ho:ho + 64, hp, kb * 128:(kb + 1) * 128],
                    qhatT[ho:ho + 64, hp, qc * 512 + off:(qc + 1) * 512],
                    start=True, stop=True)
                return sp

            def do_exps(pe):
                """exp both heads of a pending score pair -> AV-ready tuple"""
                hp, kb, spA, spB, uA, uB = pe
                off = blk_off(kb)
                ess = []
                for h, sp in ((2 * hp, spA), (2 * hp + 1, spB)):
                    if variant == "mask":
                        nc.vector.tensor_add(sp[:], sp[:], mks[kb][:])
                    es = espool.tile([128, 512], BF16, tag="es")
                    nc.scalar.activation(es[:, 0:512 - off], sp[:, off:512],
                                         AF.Exp)
                    if variant == "causal" and kb >= 4 * qc:
                        # staircase nontrivial only in the first 128 live cols
                        nc.vector.tensor_mul(
                            es[:, 0:128], es[:, 0:128],
                            pat_sb[:, kb - 4 * qc, off:off + 128])
                    ess.append(es)
                return (hp, kb, ess[0], ess[1], uA, uB)

            def do_avs(pa):
                hp, kb, esA, esB, uA, uB = pa
                off = blk_off(kb)
                for h, es, u_h in ((2 * hp, esA, uA), (2 * hp + 1, esB, uB)):
                    nc.tensor.matmul(u_h[:, off:512], v_ext[:, kb, h, :],
                                     es[:, 0:512 - off],
                                     start=(kb == kbs[0]), stop=(kb == kbs[-1]),
                                     skip_group_check=True)

            # Per-pair softmax tail: numerators+denominators are copied out
            # of PSUM as soon as the pair completes (freeing the u banks for
            # the next pair); ONE approx-reciprocal + ONE block-diag
            # broadcast matmul + ONE multiply serve both heads.
            uas, rdgs = {}, {}

            def save_pair(p, uA, uB):
                ua = uapool.tile([128, 512], F32, tag="ua",
                                 name=f"ua{qc}_{p}")
                nc.vector.tensor_copy(ua[0:64, :], uA[0:D, :])
                nc.vector.tensor_copy(ua[64:128, :], uB[0:D, :])
                gs = [gpool.tile([1, 512], F32, tag="g",
                                 name=f"g{qc}_{p}_{hm}") for hm in range(2)]
                nc.vector.tensor_copy(gs[0][:], uA[D:D + 1, :])
                nc.vector.tensor_copy(gs[1][:], uB[D:D + 1, :])
                uas[p] = ua
                return gs

            def recip_pair(p, gs):
                # single-op DVE approx reciprocal (~18 bits), written as
                # f32r directly so the broadcast matmul can consume it
                from concourse.dve_ops import (
                    RECIP_APPROX_FAST_CONSTS, RECIPROCAL_APPROX_FAST)
                c = RECIP_APPROX_FAST_CONSTS
                rdg = []
                for hm in range(2):
                    r = gpool.tile([1, 512], F32R, tag="g",
                                   name=f"rdg{qc}_{p}_{hm}")
                    nc.vector._custom_dve(
                        RECIPROCAL_APPROX_FAST, out=r[:], in0=gs[hm][:],
                        s0=c["s0"], s1=c["s1"], imm2=c["imm2"])
                    rdg.append(r)
                rdgs[p] = rdg

            def gnorm_pair(p):
                rdg, ua = rdgs[p], uas[p]
                for hm in range(2):
                    bcd = spsum.tile([64, 512], F32, tag="s",
                                     name=f"gbc{qc}_{p}_{hm}")
                    nc.tensor.matmul(bcd[:], ones64_r[:], rdg[hm][:],
                                     start=True, stop=True)
                    nc.vector.tensor_mul(
                        aT[hm * 64:(hm + 1) * 64, p,
                           qc * 512:(qc + 1) * 512],
                        ua[hm * 64:(hm + 1) * 64, :], bcd[:])

            ysb = None

            def oproj_part(oqc, part):
                """output projection for chunk oqc, jt pair `part` (0..3)"""
                nonlocal ysb
                if part == 0:
                    ysb = ypool.tile([128, NKD, 512], BF16, tag="ys",
                                     name=f"ys{oqc}")
                for jt in (2 * part, 2 * part + 1):
                    yp = proj_psum.tile([128, 512], F32, tag="proj",
                                        name=f"y{oqc}_{jt}")
                    for kc in range(2):
                        nc.tensor.matmul(
                            yp[:], wot[:, kc, jt * 128:(jt + 1) * 128],
                            aT[:, kc, oqc * 512:(oqc + 1) * 512],
                            start=(kc == 0), stop=(kc == 1))
                    if bz:
                        nc.vector.tensor_copy(ysb[:, jt, :], yp[:])
                    else:
                        nc.scalar.activation(ysb[:, jt, :], yp[:], AF.Identity,
                                             bias=bo_sb[:, jt:jt + 1])
                if part in (1, 3):
                    jl = slice(part * 2 - 2, part * 2 + 2)
                    cols = slice((part * 2 - 2) * 512, (part * 2 + 2) * 512)
                    nc.sync.dma_start(
                        yt_p.ap()[oqc * 128:(oqc + 1) * 128, cols]
                        .rearrange("p (a f) -> p a f", f=512),
                        ysb[:, jl, :])

            # Head-pair attention stream, 2-deep pipeline: iteration i emits
            # scores(i), exps(i-1), AVs(i-2).
            pend_exp = None
            pend_av = None

            def rotate(new_pe):
                nonlocal pend_exp, pend_av
                av_next = do_exps(pend_exp) if pend_exp is not None else None
                if pend_av is not None:
                    do_avs(pend_av)
                pend_av = av_next
                pend_exp = new_pe

            def flush_tail():
                nonlocal pend_exp, pend_av
                av_next = do_exps(pend_exp) if pend_exp is not None else None
                if pend_av is not None:
                    do_avs(pend_av)
                if av_next is not None:
                    do_avs(av_next)
                pend_exp = pend_av = None

            gs0 = None
            for hp in range(2):
                hA, hB = 2 * hp, 2 * hp + 1
                uA = upsum.tile([D + 1, 512], F32, tag="u",
                                name=f"u{qc}_{hA}")
                uB = upsum.tile([D + 1, 512], F32, tag="u",
                                name=f"u{qc}_{hB}")
                for i, kb in enumerate(kbs):
                    spA = score_mm(hA, kb)
                    spB = score_mm(hB, kb)
                    rotate((hp, kb, spA, spB, uA, uB))
                    if hp == 1:
                        if i == 1:
                            # pair 0's final AV flushed this iteration
                            gs0 = save_pair(0, prevA, prevB)
                        if i == 2:
                            recip_pair(0, gs0)
                        if i == min(3, nkb - 1):
                            gnorm_pair(0)
                        if oproj_pend is not None and 3 <= i <= 6:
                            oproj_part(oproj_pend, i - 3)
                prevA, prevB = uA, uB

            # flush the final exp/AV into the next projection chunk's matmul
            # stream; the remaining normalization chain goes AFTER the proj
            # vector work so the next attention chunk's qhat muls aren't
            # queued behind it.
            if qc + 1 < NQC:
                proj_chunk(qc + 1, after_first=flush_tail)
            else:
                flush_tail()
            gs1 = save_pair(1, prevA, prevB)
            recip_pair(1, gs1)
            gnorm_pair(1)
            oproj_pend = qc

        for part in range(4):
            oproj_part(oproj_pend, part)

    nc.compile()
    return nc


_PROGRAM_CACHE: dict = {}


def _get_program(variant: str, bz: bool = True) -> bass.Bass:
    key = (variant, bz)
    if key not in _PROGRAM_CACHE:
        _PROGRAM_CACHE[key] = build_program(variant, bz)
    return _PROGRAM_CACHE[key]


def _detect_variant(mask: np.ndarray) -> str:
    m = np.asarray(mask).reshape(T, T)
    if not m.any():
        return "zeros"
    tri = np.tril(np.ones((T, T), dtype=bool))
    if np.all(m[tri] == 0.0) and np.all(m[~tri] <= -1e8):
        return "causal"
    return "mask"


def _staircase_patterns() -> np.ndarray:
    kk = np.arange(128)[:, None, None]
    ai = np.arange(4)[None, :, None]
    qq = np.arange(512)[None, None, :]
    return (kk + ai * 128 <= qq).astype(BF)


def build_core_inputs(variant, x, mask, Wq, bq, Wk, bk, Wv, bv, Wo, bo, tau):
    """Host-side shard + pre-transpose + relayout + bf16 cast."""
    x = np.asarray(x, dtype=np.float32)
    Wq = np.asarray(Wq, dtype=np.float32)
    Wk = np.asarray(Wk, dtype=np.float32)
    Wv = np.asarray(Wv, dtype=np.float32)
    Wo = np.asarray(Wo, dtype=np.float32)
    bq = np.asarray(bq, dtype=np.float32)
    bk = np.asarray(bk, dtype=np.float32)
    bv = np.asarray(bv, dtype=np.float32)
    bo = np.asarray(bo, dtype=np.float32)
    tau = np.asarray(tau, dtype=np.float32).reshape(H)

    pat = _staircase_patterns() if variant == "causal" else None
    maskt = (np.ascontiguousarray(
        np.asarray(mask, dtype=np.float32).reshape(T, T).T)
        if variant == "mask" else None)

    oblk = np.zeros((128, 4, 8), dtype=BF)
    for j in range(4):
        oblk[0:64, j, 2 * j] = 1
        oblk[64:128, j, 2 * j + 1] = 1

    def dk_major(a):
        """[NKD*128, F] -> [128, NKD*F] with dk the slower free index"""
        kd, f = a.shape
        return np.ascontiguousarray(
            a.reshape(NKD, 128, f).transpose(1, 0, 2).reshape(128, NKD * f))

    in_maps = []
    for c in range(NCORES):
        b = c // 4
        h0 = (c % 4) * HPC
        sl = slice(h0 * D, (h0 + HPC) * D)
        tblk = np.zeros((8, 512), dtype=np.float32)
        for j in range(4):
            v0 = tau[h0 + 2 * j] if j < 2 else 1.0
            v1 = tau[h0 + 2 * j + 1] if j < 2 else 1.0
            tblk[2 * j, j * 128:j * 128 + 64] = v0
            tblk[2 * j + 1, j * 128 + 64:(j + 1) * 128] = v1
        bqk = np.stack([bq[sl][0:128], bq[sl][128:256],
                        bk[sl][0:128], bk[sl][128:256]], axis=1)
        xT = np.ascontiguousarray(x[b].T)  # [DM, T]
        # chunk-major [qc*128+p, dk*512+t'] = xT[dk*128+p, qc*512+t']
        xtp = np.ascontiguousarray(
            xT.reshape(NKD, 128, NQC, 512).transpose(2, 1, 0, 3)
            .reshape(NQC * 128, NKD * 512))
        wotT = np.ascontiguousarray(Wo[:, sl].T)  # [HD, DM]
        m = {
            "xt": xtp.astype(BF),
            "wqkt": dk_major(np.concatenate(
                [Wq[sl].T, Wk[sl].T], axis=1)).astype(BF),
            "wvt": dk_major(np.ascontiguousarray(Wv[sl].T)).astype(BF),
            "wot": np.ascontiguousarray(
                wotT.reshape(2, 128, DM).transpose(1, 0, 2)
                .reshape(128, 2 * DM)).astype(BF),
            "bqk": np.ascontiguousarray(bqk),
            "bvr": bv[sl].reshape(1, HD).astype(BF),
            "bo": (bo.reshape(NKD, 128).T.copy() if c % 4 == 0
                   else np.zeros((128, NKD), dtype=np.float32)),
            "tblk": tblk,
            "oblk": oblk,
        }
        if variant == "causal":
            m["pat"] = pat
        if variant == "mask":
            m["maskt"] = maskt
        in_maps.append(m)
    return in_maps


def kernel(x, mask, Wq, bq, Wk, bk, Wv, bv, Wo, bo, tau):
    variant = _detect_variant(np.asarray(mask, dtype=np.float32))
    bz = not (np.asarray(bq).any() or np.asarray(bk).any()
              or np.asarray(bv).any() or np.asarray(bo).any())
    nc = _get_program(variant, bz)
    in_maps = build_core_inputs(variant, x, mask, Wq, bq, Wk, bk,
                                Wv, bv, Wo, bo, tau)
    res = run_bass_kernel_spmd(nc, in_maps, list(range(NCORES)))
    out = np.empty((B, T, DM), dtype=np.float32)
    for b in range(B):
        acc = res.results[4 * b]["yt"].astype(np.float32)
        for c in range(4 * b + 1, 4 * b + 4):
            acc += res.results[c]["yt"].astype(np.float32)
        # [qc*128+p, jt*512+t'] -> [DM, T] -> [T, DM]
        y = acc.reshape(NQC, 128, NKD, 512).transpose(2, 1, 0, 3) \
               .reshape(DM, T)
        out[b] = y.T
    return out


# revision 10
# speedup vs baseline: 1.0460x; 1.0460x over previous
"""Trainium2 Bass kernel for CosineAttention.

Model (fp32 reference):
  q = l2norm_head(x @ Wq.T + bq), k = l2norm_head(x @ Wk.T + bk), v = x @ Wv.T + bv
  attn = softmax(tau_h * (q . k) + mask), out = (attn @ v) @ Wo.T + bo

Sharding: B*H = 2*16 = 32 (batch, head) units over 8 cores -> each core owns
one batch (b = core//4) and 4 heads. q/k/v projections are column-sharded
(rows of W), the output projection row-sharded; partial [DM, T] outputs are
summed per batch on the host (the unshard step). tau shards with heads.

All heavy matmuls run with bf16 operands accumulating in fp32 PSUM. All
layout transposes AND relayouts are done on the HOST (free): every DMA the
kernel issues is a plain contiguous [128, N] block, so descriptor
generation is cheap (~0.6us issue each, ~10 DMAs total). Scores are
computed transposed: S^T[k, q] = k-hat @ q-hat^T so softmax needs no max
subtraction (|tau*cos| <= tau) and exp(S^T) feeds the AV matmul directly as
the moving operand. An appended ones-column on V gives the softmax
denominator for free in PSUM row 64. Causal masks are lowered to skipping
fully masked S^T blocks plus 0/1 staircase multiplies on diagonal blocks.

Perf structure (v2):
- A 16-matmul dummy accumulation group + dummy Ln activation run during the
  DMA wait: they warm the PE HAM clock gate (1.2 -> 2.4 GHz) and load the
  ACT table set off the critical path.
- Attention processes heads in PAIRS with a 2-deep software pipeline:
  iteration i issues score(i) [adjacent K=64 matmuls on disjoint array row
  groups -> they run concurrently], exp(i-1) on Scalar, and AV(i-2) on the
  PE. The AV's exp finished a full iteration earlier, so at most one PE
  instruction per iteration blocks on a semaphore.
- PSUM: scores cycle 4 banks, u accumulators 2, projection/output 2; norm
  sums and the warmup share the projection pool, the denominator broadcast
  borrows a score bank at pair tails.
- Softmax denominator: both heads' sums sit in a [2,512] tile, one
  single-op DVE approx reciprocal (~18 bits), then ONE block-diagonal K=2
  matmul broadcasts both rows and ONE full-width multiply normalizes the
  pair.
- q/k-norm Square runs on Scalar straight from PSUM while the bias-free qb
  copy runs on DVE, balancing the two engines.
"""

import numpy as np
from contextlib import ExitStack

import ml_dtypes

import concourse.bass as bass
import concourse.mybir as mybir
import concourse.tile as tile
from concourse import bacc
from concourse.bass_utils import run_bass_kernel_spmd

B, T, DM, H = 2, 2048, 1024, 16
D = 64
NCORES = 8
HPC = 4            # heads per core
HD = HPC * D       # 256 head dims per core
NT = T // 128      # 16 k-blocks / T-tiles
NQC = T // 512     # 4 q-chunks
NKD = DM // 128    # 8 contraction chunks of DM
F32 = mybir.dt.float32
F32R = mybir.dt.float32r
BF16 = mybir.dt.bfloat16
AF = mybir.ActivationFunctionType
BF = ml_dtypes.bfloat16


def build_program(variant: str, bz: bool) -> bass.Bass:
    """variant: 'causal' | 'zeros' | 'mask'; bz: all biases are zero"""
    assert variant in ("causal", "zeros", "mask")
    nc = bacc.Bacc("TRN2", target_bir_lowering=False, debug=False,
                   num_devices=NCORES)

    # all big tensors pre-permuted on host so DMAs are contiguous blocks
    xt_p = nc.declare_dram_parameter("xt", [NQC * 128, NKD * 512], BF16,
                                     isOutput=False)
    wqkt_p = nc.declare_dram_parameter("wqkt", [128, NKD * 512], BF16,
                                       isOutput=False)
    wvt_p = nc.declare_dram_parameter("wvt", [128, NKD * HD], BF16,
                                      isOutput=False)
    wot_p = nc.declare_dram_parameter("wot", [128, 2 * DM], BF16,
                                      isOutput=False)
    bqk_p = nc.declare_dram_parameter("bqk", [128, 4], F32, isOutput=False)
    bvr_p = nc.declare_dram_parameter("bvr", [1, HD], BF16, isOutput=False)
    bo_p = nc.declare_dram_parameter("bo", [128, NKD], F32, isOutput=False)
    tblk_p = nc.declare_dram_parameter("tblk", [8, 512], F32, isOutput=False)
    oblk_p = nc.declare_dram_parameter("oblk", [128, 4, 8], BF16, isOutput=False)
    if variant == "causal":
        pat_p = nc.declare_dram_parameter("pat", [128, 4, 512], BF16, isOutput=False)
    if variant == "mask":
        mt_p = nc.declare_dram_parameter("maskt", [T, T], F32, isOutput=False)
    yt_p = nc.declare_dram_parameter("yt", [NQC * 128, NKD * 512], BF16,
                                     isOutput=True)

    with tile.TileContext(nc) as tc, ExitStack() as top:
        const = top.enter_context(tc.tile_pool(name="const", bufs=1))
        wts = top.enter_context(tc.tile_pool(name="wts", bufs=1))
        acts = top.enter_context(tc.tile_pool(name="acts", bufs=1))

        # ---- weights / activations resident in SBUF ----
        wqkt = wts.tile([128, NKD, 2 * HD], BF16, tag="wqkt")
        wvt = wts.tile([128, NKD, HD], BF16, tag="wvt")
        wot = wts.tile([128, 2, DM], BF16, tag="wot")
        # chunk-major so each chunk's DMA is one contiguous block
        xts = wts.tile([128, NQC, NKD, 512], BF16, tag="xts")

        qhatT = acts.tile([128, 2, T], BF16, tag="qhatT")
        khatT = acts.tile([128, 2, T], BF16, tag="khatT")
        v_ext = acts.tile([128, NT, HPC, D + 1], BF16, tag="v_ext")
        aT = acts.tile([128, 2, T], BF16, tag="aT")

        # ---- DMA: one contiguous transfer per tensor / chunk ----
        nc.sync.dma_start(wqkt[:],
                          wqkt_p.ap().rearrange("p (a f) -> p a f", f=512))
        nc.sync.dma_start(xts[:, 0, :, :],
                          xt_p.ap()[0:128, :].rearrange("p (a f) -> p a f",
                                                        f=512))
        nc.sync.dma_start(wvt[:],
                          wvt_p.ap().rearrange("p (a f) -> p a f", f=HD))
        if not bz:
            bqk = const.tile([128, 4], F32)
            nc.sync.dma_start(bqk[:], bqk_p.ap())
            bvr = const.tile([1, HD], BF16)
            nc.sync.dma_start(bvr[:], bvr_p.ap())
        tblk_f = const.tile([8, 512], F32)
        nc.sync.dma_start(tblk_f[:], tblk_p.ap())
        oblk = const.tile([128, 4, 8], BF16)
        nc.sync.dma_start(oblk[:], oblk_p.ap())
        if variant == "causal":
            pat_sb = const.tile([128, 4, 512], BF16)
            nc.sync.dma_start(pat_sb[:], pat_p.ap())
        for tc_i in range(1, NQC):
            nc.sync.dma_start(
                xts[:, tc_i, :, :],
                xt_p.ap()[tc_i * 128:(tc_i + 1) * 128, :]
                .rearrange("p (a f) -> p a f", f=512))
        nc.sync.dma_start(wot[:],
                          wot_p.ap().rearrange("p (a f) -> p a f", f=DM))
        if not bz:
            bo_sb = const.tile([128, NKD], F32)
            nc.sync.dma_start(bo_sb[:], bo_p.ap())

        # ---- small constants ----
        ones_f = const.tile([1, 128], F32)
        nc.vector.memset(ones_f[:], 1.0)
        ones_b = const.tile([1, 128], BF16)
        nc.vector.tensor_copy(ones_b[:], ones_f[:])
        ones64_r = const.tile([1, 64], F32R)
        nc.vector.tensor_copy(ones64_r[:], ones_f[:, 0:64])
        tblk_r = const.tile([8, 512], F32R)
        nc.vector.tensor_copy(tblk_r[:], tblk_f[:])
        onesv_f = const.tile([128, NT * HPC], F32)
        nc.vector.memset(onesv_f[:], 1.0)
        # ones column of v_ext (softmax denominator trick)
        nc.vector.tensor_copy(
            v_ext[:, :, :, D:D + 1],
            onesv_f[:].rearrange("p (a b c) -> p a b c", a=NT, b=HPC))
        # warm-up operands
        wrm = const.tile([128, 512], BF16)
        nc.vector.memset(wrm[:], 0.001)
        lnw = const.tile([1, 8], F32)

        # ---- pools ----
        uapool = top.enter_context(tc.tile_pool(name="uapool", bufs=2))
        gpool = top.enter_context(tc.tile_pool(name="gpool", bufs=3))
        qkpool = top.enter_context(tc.tile_pool(name="qkpool", bufs=2))
        sqpool = top.enter_context(tc.tile_pool(name="sqpool", bufs=2))
        espool = top.enter_context(tc.tile_pool(name="espool", bufs=8))
        ypool = top.enter_context(tc.tile_pool(name="ypool", bufs=2))
        if variant == "mask":
            mpool = top.enter_context(tc.tile_pool(name="mpool", bufs=NT))
        # PSUM budget (8 banks): scores/bcast 4 + u 2 + proj/y/norm 2
        spsum = top.enter_context(
            tc.tile_pool(name="spsum", bufs=4, space="PSUM"))
        upsum = top.enter_context(
            tc.tile_pool(name="upsum", bufs=2, space="PSUM"))
        proj_psum = top.enter_context(
            tc.tile_pool(name="proj_psum", bufs=2, space="PSUM"))

        # ---- warm-up: PE HAM + ACT table load, during the DMA wait ----
        nc.scalar.activation(lnw[:], ones_f[:, 0:8], AF.Ln)
        wps = proj_psum.tile([128, 512], F32, tag="proj", name="warm")
        for i in range(16):
            nc.tensor.matmul(wps[:], wrm[:, 0:128], wrm[:],
                             start=(i == 0), stop=(i == 15))

        def proj_chunk(tcx, after_first=None):
            """project tokens [tcx*512, (tcx+1)*512) -> qhatT/khatT/v_ext.

            after_first, if given, is called after the first matmul group is
            emitted — used to flush the previous attention chunk's trailing
            exp/AV so its exp lands early in the scalar queue while the PE
            already has projection work."""
            ts = slice(tcx * 512, (tcx + 1) * 512)
            qb = qkpool.tile([128, 4, 512], F32, tag="qb", name=f"qb{tcx}")
            sq = sqpool.tile([128, 4, 512], BF16, tag="sq", name=f"sq{tcx}")

            def v_tile(tt):
                t = tcx * 4 + tt
                pv = proj_psum.tile([128, HD], F32, tag="proj",
                                    name=f"pv{t}")
                for dk in range(NKD):
                    nc.tensor.matmul(pv[:],
                                     xts[:, tcx, dk, tt * 128:(tt + 1) * 128],
                                     wvt[:, dk, :],
                                     start=(dk == 0), stop=(bz and dk == NKD - 1))
                if not bz:
                    nc.tensor.matmul(pv[:], ones_b[:], bvr[:],
                                     start=False, stop=True)
                nc.vector.tensor_copy(v_ext[:, t, :, 0:D],
                                      pv[:].rearrange("p (h d) -> p h d", h=HPC))

            for j in range(4):
                pj = proj_psum.tile([128, 512], F32, tag="proj",
                                    name=f"pj{tcx}_{j}")
                for dk in range(NKD):
                    nc.tensor.matmul(pj[:],
                                     wqkt[:, dk, j * 128:(j + 1) * 128],
                                     xts[:, tcx, dk, :],
                                     start=(dk == 0), stop=(dk == NKD - 1))
                if j == 0 and after_first is not None:
                    after_first()
                if tcx == 0 and j >= 2:
                    # chunk 0 has no attention stream to hide latency; V
                    # tiles are the only independent PE work
                    v_tile(j - 2)
                # Identity is in every ACT table set (Square is NOT — using
                # it forces a ~2.7us table reload per chunk)
                if bz:
                    nc.scalar.activation(qb[:, j, :], pj[:], AF.Identity)
                else:
                    nc.scalar.activation(qb[:, j, :], pj[:], AF.Identity,
                                         bias=bqk[:, j:j + 1])
                nc.vector.tensor_mul(sq[:, j, :], qb[:, j, :], qb[:, j, :])

            if tcx != 0:
                v_tile(0)
                v_tile(1)
            else:
                v_tile(2)
            ns = proj_psum.tile([8, 512], F32, tag="proj", name=f"ns{tcx}")
            for j in range(4):
                nc.tensor.matmul(ns[:], oblk[:, j, :], sq[:, j, :],
                                 start=(j == 0), stop=(j == 3))
            # rsqrt = exp(-0.5*ln): short chain, both on Scalar; the Exp
            # writes f32r directly so no cast sits before the broadcasts
            sn = gpool.tile([8, 512], F32, tag="g", name=f"sn{tcx}")
            nc.scalar.activation(sn[:], ns[:], AF.Ln)
            rr = gpool.tile([8, 512], F32R, tag="g", name=f"rr{tcx}")
            nc.scalar.activation(rr[:], sn[:], AF.Exp, scale=-0.5)

            def bcast(j):
                bcp = spsum.tile([128, 512], F32, tag="s",
                                 name=f"bcp{tcx}_{j}")
                nc.tensor.matmul(bcp[:], tblk_r[:, j * 128:(j + 1) * 128],
                                 rr[:], start=True, stop=True)
                dst = qhatT if j < 2 else khatT
                nc.vector.tensor_mul(dst[:, j % 2, ts], qb[:, j, :], bcp[:])

            if tcx != 0:
                v_tile(2)
                bcast(0)
                bcast(1)
                v_tile(3)
            else:
                v_tile(3)
                bcast(0)
                bcast(1)
            bcast(2)
            bcast(3)

        proj_chunk(0)
        oproj_pend = None

        for qc in range(NQC):
            kbs = list(range(4 * qc + 4)) if variant == "causal" else list(range(NT))
            nkb = len(kbs)
            mks = {}
            if variant == "mask":
                for kb in kbs:
                    mk = mpool.tile([128, 512], F32, tag="mk",
                                    name=f"mk{qc}_{kb}")
                    nc.sync.dma_start(
                        mk[:], mt_p.ap()[kb * 128:(kb + 1) * 128,
                                         qc * 512:(qc + 1) * 512])
                    mks[kb] = mk

            def blk_off(kb):
                """first possibly-unmasked column of this S^T block"""
                if variant != "causal":
                    return 0
                ai = kb - 4 * qc
                return ai * 128 if ai > 0 else 0

            def score_mm(h, kb):
                hp, ho = h // 2, (h % 2) * 64
                off = blk_off(kb)
                sp = spsum.tile([128, 512], F32, tag="s",
                                name=f"s{qc}_{kb}_{h}")
                nc.tensor.matmul(
                    sp[:, off:512],
                    khatT[ho:ho + 64, hp, kb * 128:(kb + 1) * 128],
                    qhatT[ho:ho + 64, hp, qc * 512 + off:(qc + 1) * 512],
                    start=True, stop=True)
                return sp

            def do_exps(pe):
                """exp both heads of a pending score pair -> AV-ready tuple"""
                hp, kb, spA, spB, uA, uB = pe
                off = blk_off(kb)
                ess = []
                for h, sp in ((2 * hp, spA), (2 * hp + 1, spB)):
                    if variant == "mask":
                        nc.vector.tensor_add(sp[:], sp[:], mks[kb][:])
                    es = espool.tile([128, 512], BF16, tag="es")
                    nc.scalar.activation(es[:, 0:512 - off], sp[:, off:512],
                                         AF.Exp)
                    if variant == "causal" and kb >= 4 * qc:
                        # staircase nontrivial only in the first 128 live cols
                        nc.vector.tensor_mul(
                            es[:, 0:128], es[:, 0:128],
                            pat_sb[:, kb - 4 * qc, off:off + 128])
                    ess.append(es)
                return (hp, kb, ess[0], ess[1], uA, uB)

            def do_avs(pa):
                hp, kb, esA, esB, uA, uB = pa
                off = blk_off(kb)
                for h, es, u_h in ((2 * hp, esA, uA), (2 * hp + 1, esB, uB)):
                    nc.tensor.matmul(u_h[:, off:512], v_ext[:, kb, h, :],
                                     es[:, 0:512 - off],
                                     start=(kb == kbs[0]), stop=(kb == kbs[-1]),
                                     skip_group_check=True)

            # Per-pair softmax tail: numerators+denominators are copied out
            # of PSUM as soon as the pair completes (freeing the u banks for
            # the next pair); ONE approx-reciprocal + ONE block-diag
            # broadcast matmul + ONE multiply serve both heads.
            uas, rdgs = {}, {}

            def save_pair(p, uA, uB):
                ua = uapool.tile([128, 512], F32, tag="ua",
                                 name=f"ua{qc}_{p}")
                nc.vector.tensor_copy(ua[0:64, :], uA[0:D, :])
                nc.vector.tensor_copy(ua[64:128, :], uB[0:D, :])
                gs = [gpool.tile([1, 512], F32, tag="g",
                                 name=f"g{qc}_{p}_{hm}") for hm in range(2)]
                nc.vector.tensor_copy(gs[0][:], uA[D:D + 1, :])
                nc.vector.tensor_copy(gs[1][:], uB[D:D + 1, :])
                uas[p] = ua
                return gs

            def recip_pair(p, gs):
                # single-op DVE approx reciprocal (~18 bits), written as
                # f32r directly so the broadcast matmul can consume it
                from concourse.dve_ops import (
                    RECIP_APPROX_FAST_CONSTS, RECIPROCAL_APPROX_FAST)
                c = RECIP_APPROX_FAST_CONSTS
                rdg = []
                for hm in range(2):
                    r = gpool.tile([1, 512], F32R, tag="g",
                                   name=f"rdg{qc}_{p}_{hm}")
                    nc.vector._custom_dve(
                        RECIPROCAL_APPROX_FAST, out=r[:], in0=gs[hm][:],
                        s0=c["s0"], s1=c["s1"], imm2=c["imm2"])
                    rdg.append(r)
                rdgs[p] = rdg

            def gnorm_pair(p):
                rdg, ua = rdgs[p], uas[p]
                for hm in range(2):
                    bcd = spsum.tile([64, 512], F32, tag="s",
                                     name=f"gbc{qc}_{p}_{hm}")
                    nc.tensor.matmul(bcd[:], ones64_r[:], rdg[hm][:],
                                     start=True, stop=True)
                    nc.vector.tensor_mul(
                        aT[hm * 64:(hm + 1) * 64, p,
                           qc * 512:(qc + 1) * 512],
                        ua[hm * 64:(hm + 1) * 64, :], bcd[:])

            ysb = None

            def oproj_part(oqc, part):
                """output projection for chunk oqc, jt pair `part` (0..3)"""
                nonlocal ysb
                if part == 0:
                    ysb = ypool.tile([128, NKD, 512], BF16, tag="ys",
                                     name=f"ys{oqc}")
                for jt in (2 * part, 2 * part + 1):
                    yp = proj_psum.tile([128, 512], F32, tag="proj",
                                        name=f"y{oqc}_{jt}")
                    for kc in range(2):
                        nc.tensor.matmul(
                            yp[:], wot[:, kc, jt * 128:(jt + 1) * 128],
                            aT[:, kc, oqc * 512:(oqc + 1) * 512],
                            start=(kc == 0), stop=(kc == 1))
                    if bz:
                        nc.vector.tensor_copy(ysb[:, jt, :], yp[:])
                    else:
                        nc.scalar.activation(ysb[:, jt, :], yp[:], AF.Identity,
                                             bias=bo_sb[:, jt:jt + 1])
                if part in (1, 3):
                    jl = slice(part * 2 - 2, part * 2 + 2)
                    cols = slice((part * 2 - 2) * 512, (part * 2 + 2) * 512)
                    nc.sync.dma_start(
                        yt_p.ap()[oqc * 128:(oqc + 1) * 128, cols]
                        .rearrange("p (a f) -> p a f", f=512),
                        ysb[:, jl, :])

            # Head-pair attention stream, 2-deep pipeline: iteration i emits
            # scores(i), exps(i-1), AVs(i-2).
            pend_exp = None
            pend_av = None

            def rotate(new_pe):
                nonlocal pend_exp, pend_av
                av_next = do_exps(pend_exp) if pend_exp is not None else None
                if pend_av is not None:
                    do_avs(pend_av)
                pend_av = av_next
                pend_exp = new_pe

            def flush_tail():
                nonlocal pend_exp, pend_av
                av_next = do_exps(pend_exp) if pend_exp is not None else None
                if pend_av is not None:
                    do_avs(pend_av)
                if av_next is not None:
                    do_avs(av_next)
                pend_exp = pend_av = None

            gs0 = None
            for hp in range(2):
                hA, hB = 2 * hp, 2 * hp + 1
                uA = upsum.tile([D + 1, 512], F32, tag="u",
                                name=f"u{qc}_{hA}")
                uB = upsum.tile([D + 1, 512], F32, tag="u",
                                name=f"u{qc}_{hB}")
                for i, kb in enumerate(kbs):
                    spA = score_mm(hA, kb)
                    spB = score_mm(hB, kb)
                    rotate((hp, kb, spA, spB, uA, uB))
                    if hp == 1:
                        if i == 1:
                            # pair 0's final AV flushed this iteration
                            gs0 = save_pair(0, prevA, prevB)
                        if i == 2:
                            recip_pair(0, gs0)
                        if i == min(3, nkb - 1):
                            gnorm_pair(0)
                        if oproj_pend is not None and 3 <= i <= 6:
                            oproj_part(oproj_pend, i - 3)
                prevA, prevB = uA, uB

            # flush the final exp/AV into the next projection chunk's matmul
            # stream; the remaining normalization chain goes AFTER the proj
            # vector work so the next attention chunk's qhat muls aren't
            # queued behind it.
            if qc + 1 < NQC:
                proj_chunk(qc + 1, after_first=flush_tail)
            else:
                flush_tail()
            gs1 = save_pair(1, prevA, prevB)
            recip_pair(1, gs1)
            gnorm_pair(1)
            oproj_pend = qc

        for part in range(4):
            oproj_part(oproj_pend, part)

    nc.compile()
    return nc


_PROGRAM_CACHE: dict = {}


def _get_program(variant: str, bz: bool = True) -> bass.Bass:
    key = (variant, bz)
    if key not in _PROGRAM_CACHE:
        _PROGRAM_CACHE[key] = build_program(variant, bz)
    return _PROGRAM_CACHE[key]


def _detect_variant(mask: np.ndarray) -> str:
    m = np.asarray(mask).reshape(T, T)
    if not m.any():
        return "zeros"
    tri = np.tril(np.ones((T, T), dtype=bool))
    if np.all(m[tri] == 0.0) and np.all(m[~tri] <= -1e8):
        return "causal"
    return "mask"


def _staircase_patterns() -> np.ndarray:
    kk = np.arange(128)[:, None, None]
    ai = np.arange(4)[None, :, None]
    qq = np.arange(512)[None, None, :]
    return (kk + ai * 128 <= qq).astype(BF)


def build_core_inputs(variant, x, mask, Wq, bq, Wk, bk, Wv, bv, Wo, bo, tau):
    """Host-side shard + pre-transpose + relayout + bf16 cast."""
    x = np.asarray(x, dtype=np.float32)
    Wq = np.asarray(Wq, dtype=np.float32)
    Wk = np.asarray(Wk, dtype=np.float32)
    Wv = np.asarray(Wv, dtype=np.float32)
    Wo = np.asarray(Wo, dtype=np.float32)
    bq = np.asarray(bq, dtype=np.float32)
    bk = np.asarray(bk, dtype=np.float32)
    bv = np.asarray(bv, dtype=np.float32)
    bo = np.asarray(bo, dtype=np.float32)
    tau = np.asarray(tau, dtype=np.float32).reshape(H)

    pat = _staircase_patterns() if variant == "causal" else None
    maskt = (np.ascontiguousarray(
        np.asarray(mask, dtype=np.float32).reshape(T, T).T)
        if variant == "mask" else None)

    oblk = np.zeros((128, 4, 8), dtype=BF)
    for j in range(4):
        oblk[0:64, j, 2 * j] = 1
        oblk[64:128, j, 2 * j + 1] = 1

    def dk_major(a):
        """[NKD*128, F] -> [128, NKD*F] with dk the slower free index"""
        kd, f = a.shape
        return np.ascontiguousarray(
            a.reshape(NKD, 128, f).transpose(1, 0, 2).reshape(128, NKD * f))

    in_maps = []
    for c in range(NCORES):
        b = c // 4
        h0 = (c % 4) * HPC
        sl = slice(h0 * D, (h0 + HPC) * D)
        tblk = np.zeros((8, 512), dtype=np.float32)
        for j in range(4):
            v0 = tau[h0 + 2 * j] if j < 2 else 1.0
            v1 = tau[h0 + 2 * j + 1] if j < 2 else 1.0
            tblk[2 * j, j * 128:j * 128 + 64] = v0
            tblk[2 * j + 1, j * 128 + 64:(j + 1) * 128] = v1
        bqk = np.stack([bq[sl][0:128], bq[sl][128:256],
                        bk[sl][0:128], bk[sl][128:256]], axis=1)
        xT = np.ascontiguousarray(x[b].T)  # [DM, T]
        # chunk-major [qc*128+p, dk*512+t'] = xT[dk*128+p, qc*512+t']
        xtp = np.ascontiguousarray(
            xT.reshape(NKD, 128, NQC, 512).transpose(2, 1, 0, 3)
            .reshape(NQC * 128, NKD * 512))
        wotT = np.ascontiguousarray(Wo[:, sl].T)  # [HD, DM]
        m = {
            "xt": xtp.astype(BF),
            "wqkt": dk_major(np.concatenate(
                [Wq[sl].T, Wk[sl].T], axis=1)).astype(BF),
            "wvt": dk_major(np.ascontiguousarray(Wv[sl].T)).astype(BF),
            "wot": np.ascontiguousarray(
                wotT.reshape(2, 128, DM).transpose(1, 0, 2)
                .reshape(128, 2 * DM)).astype(BF),
            "bqk": np.ascontiguousarray(bqk),
            "bvr": bv[sl].reshape(1, HD).astype(BF),
            "bo": (bo.reshape(NKD, 128).T.copy() if c % 4 == 0
                   else np.zeros((128, NKD), dtype=np.float32)),
            "tblk": tblk,
            "oblk": oblk,
        }
        if variant == "causal":
            m["pat"] = pat
        if variant == "mask":
            m["maskt"] = maskt
        in_maps.append(m)
    return in_maps


def kernel(x, mask, Wq, bq, Wk, bk, Wv, bv, Wo, bo, tau):
    variant = _detect_variant(np.asarray(mask, dtype=np.float32))
    bz = not (np.asarray(bq).any() or np.asarray(bk).any()
              or np.asarray(bv).any() or np.asarray(bo).any())
    nc = _get_program(variant, bz)
    in_maps = build_core_inputs(variant, x, mask, Wq, bq, Wk, bk,
                                Wv, bv, Wo, bo, tau)
    res = run_bass_kernel_spmd(nc, in_maps, list(range(NCORES)))
    out = np.empty((B, T, DM), dtype=np.float32)
    for b in range(B):
        acc = res.results[4 * b]["yt"].astype(np.float32)
        for c in range(4 * b + 1, 4 * b + 4):
            acc += res.results[c]["yt"].astype(np.float32)
        # [qc*128+p, jt*512+t'] -> [DM, T] -> [T, DM]
        y = acc.reshape(NQC, 128, NKD, 512).transpose(2, 1, 0, 3) \
               .reshape(DM, T)
        out[b] = y.T
    return out


# revision 38
# speedup vs baseline: 1.1495x; 1.0990x over previous
"""Trainium2 Bass kernel for CosineAttention.

Model (fp32 reference):
  q = l2norm_head(x @ Wq.T + bq), k = l2norm_head(x @ Wk.T + bk), v = x @ Wv.T + bv
  attn = softmax(tau_h * (q . k) + mask), out = (attn @ v) @ Wo.T + bo

Sharding: B*H = 2*16 = 32 (batch, head) units over 8 cores -> each core owns
one batch (b = core//4) and 4 heads. q/k/v projections are column-sharded
(rows of W), the output projection row-sharded; partial [DM, T] outputs are
summed per batch on the host (the unshard step). tau shards with heads.

All heavy matmuls run bf16 operands accumulating in fp32 PSUM. All layout
transposes AND relayouts happen on the HOST (free): every DMA is a plain
contiguous [128, N] block (~0.6us issue each, ~10 DMAs total). Scores are
computed transposed: S^T[k, q] = k-hat @ q-hat^T so softmax needs no max
subtraction (|tau*cos| <= tau) and exp(S^T) feeds the AV matmul directly as
the moving operand. An appended ones-column on V gives the softmax
denominator for free in PSUM row 64. Causal masks are lowered to skipping
fully masked S^T blocks plus 0/1 staircase multiplies on diagonal blocks.

Performance structure (the kernel is paced by the Scalar-engine exp stream,
~1.0-1.1us per key-block; everything else hides under or between it):

- Warm-up: a dummy 16-matmul accumulation group plus a dummy Ln activation
  run during the initial DMA wait, warming the PE HAM clock gate
  (1.2 -> 2.4 GHz) and loading the ACT table set off the critical path.
  Only Identity/Ln/Exp are ever used on Scalar -- all in ONE table set
  (Square is not: using it forces ~2.7us table reloads every chunk).

- Attention is ONE global software pipeline across all q-chunks. Iteration
  g emits: a score PAIR (both heads of a pair matmul K=64 blocks into one
  2-bank PSUM tile; their disjoint array row-groups run concurrently),
  any scheduled softmax-tail stitches, exp(g-1) (ONE activation covering
  both heads, reading [128, 2, 512-off] across the two banks), AV(g-2)
  (the exp finished a full iteration earlier, so AV matmuls do not wait),
  then a paced quantum of filler.

- Filler = the next chunk's projection pieces (half-size matmul
  accumulation groups; their Scalar/DVE continuations trail by one piece
  so cross-queue emission order stays deadlock-free) plus, at lower
  priority, finished chunks' output projections (no deadline, paced toward
  the Scalar-heavy late chunks). This keeps PE array duty high enough that
  the HAM clock gate stays warm, and hides most of the projection phases
  inside attention's Scalar-bound slack.

- Chunk tails (numerator/denominator copies out of PSUM, one single-op DVE
  approx-reciprocal per head (~18 bits), denominator broadcast matmuls and
  the normalize multiplies) are scheduled as stitches +3..+5 iterations
  into the FOLLOWING chunk, so the PE never drains at a chunk boundary.

- PSUM: 3 shared 2-bank buffers rotate scores/projection/output tiles,
  2 banks hold the pair's AV accumulators (row 64 = denominator).
"""

import numpy as np
from contextlib import ExitStack

import ml_dtypes

import concourse.bass as bass
import concourse.mybir as mybir
import concourse.tile as tile
from concourse import bacc
from concourse.bass_utils import run_bass_kernel_spmd

B, T, DM, H = 2, 2048, 1024, 16
D = 64
NCORES = 8
HPC = 4            # heads per core
HD = HPC * D       # 256 head dims per core
NT = T // 128      # 16 k-blocks / T-tiles
NQC = T // 512     # 4 q-chunks
NKD = DM // 128    # 8 contraction chunks of DM
F32 = mybir.dt.float32
F32R = mybir.dt.float32r
BF16 = mybir.dt.bfloat16
AF = mybir.ActivationFunctionType
BF = ml_dtypes.bfloat16


def build_program(variant: str, bz: bool) -> bass.Bass:
    """variant: 'causal' | 'zeros' | 'mask'; bz: all biases are zero"""
    assert variant in ("causal", "zeros", "mask")
    nc = bacc.Bacc("TRN2", target_bir_lowering=False, debug=False,
                   num_devices=NCORES)

    # all big tensors pre-permuted on host so DMAs are contiguous blocks
    xt_p = nc.declare_dram_parameter("xt", [NQC * 128, NKD * 512], BF16,
                                     isOutput=False)
    wqkt_p = nc.declare_dram_parameter("wqkt", [128, NKD * 512], BF16,
                                       isOutput=False)
    wvt_p = nc.declare_dram_parameter("wvt", [128, NKD * HD], BF16,
                                      isOutput=False)
    wot_p = nc.declare_dram_parameter("wot", [128, 2 * DM], BF16,
                                      isOutput=False)
    bqk_p = nc.declare_dram_parameter("bqk", [128, 4], F32, isOutput=False)
    bvr_p = nc.declare_dram_parameter("bvr", [1, HD], BF16, isOutput=False)
    bo_p = nc.declare_dram_parameter("bo", [128, NKD], F32, isOutput=False)
    tblk_p = nc.declare_dram_parameter("tblk", [8, 512], F32, isOutput=False)
    oblk_p = nc.declare_dram_parameter("oblk", [128, 4, 8], BF16, isOutput=False)
    if variant == "causal":
        pat_p = nc.declare_dram_parameter("pat", [128, 4, 512], BF16, isOutput=False)
    if variant == "mask":
        mt_p = nc.declare_dram_parameter("maskt", [T, T], F32, isOutput=False)
    yt_p = nc.declare_dram_parameter("yt", [NQC * 128, NKD * 512], BF16,
                                     isOutput=True)

    with tile.TileContext(nc) as tc, ExitStack() as top:
        const = top.enter_context(tc.tile_pool(name="const", bufs=1))
        wts = top.enter_context(tc.tile_pool(name="wts", bufs=1))
        acts = top.enter_context(tc.tile_pool(name="acts", bufs=1))

        # ---- weights / activations resident in SBUF ----
        wqkt = wts.tile([128, NKD, 2 * HD], BF16, tag="wqkt")
        wvt = wts.tile([128, NKD, HD], BF16, tag="wvt")
        wot = wts.tile([128, 2, DM], BF16, tag="wot")
        # chunk-major so each chunk's DMA is one contiguous block
        xts = wts.tile([128, NQC, NKD, 512], BF16, tag="xts")

        qhatT = acts.tile([128, 2, T], BF16, tag="qhatT")
        khatT = acts.tile([128, 2, T], BF16, tag="khatT")
        v_ext = acts.tile([128, NT, HPC, D + 1], BF16, tag="v_ext")
        aT = acts.tile([128, 2, T], BF16, tag="aT")

        # ---- DMA: one contiguous transfer per tensor / chunk ----
        nc.sync.dma_start(wqkt[:],
                          wqkt_p.ap().rearrange("p (a f) -> p a f", f=512))
        nc.sync.dma_start(xts[:, 0, 0:4, :],
                          xt_p.ap()[0:128, 0:2048]
                          .rearrange("p (a f) -> p a f", f=512))
        nc.sync.dma_start(xts[:, 0, 4:8, :],
                          xt_p.ap()[0:128, 2048:4096]
                          .rearrange("p (a f) -> p a f", f=512))
        nc.sync.dma_start(wvt[:],
                          wvt_p.ap().rearrange("p (a f) -> p a f", f=HD))
        if not bz:
            bqk = const.tile([128, 4], F32)
            nc.sync.dma_start(bqk[:], bqk_p.ap())
            bvr = const.tile([1, HD], BF16)
            nc.sync.dma_start(bvr[:], bvr_p.ap())
        tblk_f = const.tile([8, 512], F32)
        nc.sync.dma_start(tblk_f[:], tblk_p.ap())
        oblk = const.tile([128, 4, 8], BF16)
        nc.sync.dma_start(oblk[:], oblk_p.ap())
        if variant == "causal":
            pat_sb = const.tile([128, 4, 512], BF16)
            nc.sync.dma_start(pat_sb[:], pat_p.ap())
        for tc_i in range(1, NQC):
            nc.sync.dma_start(
                xts[:, tc_i, :, :],
                xt_p.ap()[tc_i * 128:(tc_i + 1) * 128, :]
                .rearrange("p (a f) -> p a f", f=512))
        nc.sync.dma_start(wot[:],
                          wot_p.ap().rearrange("p (a f) -> p a f", f=DM))
        if not bz:
            bo_sb = const.tile([128, NKD], F32)
            nc.sync.dma_start(bo_sb[:], bo_p.ap())

        # ---- small constants ----
        ones_f = const.tile([1, 128], F32)
        nc.vector.memset(ones_f[:], 1.0)
        ones_b = const.tile([1, 128], BF16)
        nc.vector.tensor_copy(ones_b[:], ones_f[:])
        ones64_b = const.tile([1, 64], BF16)
        nc.vector.tensor_copy(ones64_b[:], ones_f[:, 0:64])
        tblk_r = const.tile([8, 512], F32R)
        nc.vector.tensor_copy(tblk_r[:], tblk_f[:])
        onesv_f = const.tile([128, NT * HPC], F32)
        nc.vector.memset(onesv_f[:], 1.0)
        # ones column of v_ext (softmax denominator trick)
        nc.vector.tensor_copy(
            v_ext[:, :, :, D:D + 1],
            onesv_f[:].rearrange("p (a b c) -> p a b c", a=NT, b=HPC))
        # warm-up operands
        wrm = const.tile([128, 512], BF16)
        nc.vector.memset(wrm[:], 0.001)
        lnw = const.tile([1, 8], F32)

        # ---- pools ----
        uapool = top.enter_context(tc.tile_pool(name="uapool", bufs=2))
        gpool = top.enter_context(tc.tile_pool(name="gpool", bufs=6))
        qkpool = top.enter_context(tc.tile_pool(name="qkpool", bufs=2))
        sqpool = top.enter_context(tc.tile_pool(name="sqpool", bufs=2))
        espool = top.enter_context(tc.tile_pool(name="espool", bufs=8))
        ypool = top.enter_context(tc.tile_pool(name="ypool", bufs=2))
        if variant == "mask":
            mpool = top.enter_context(tc.tile_pool(name="mpool", bufs=NT))
        # PSUM budget (8 banks): score pairs 2x2 + u 2 + proj/y/misc 2.
        # Both heads of a pair score into ONE 2-bank tile, so one exp
        # activation covers both and both scores share one release gate.
        spool = top.enter_context(
            tc.tile_pool(name="spool", bufs=3, space="PSUM"))
        upsum = top.enter_context(
            tc.tile_pool(name="upsum", bufs=2, space="PSUM"))
        proj_psum = spool

        # ---- warm-up: PE HAM + ACT table load, during the DMA wait ----
        nc.scalar.activation(lnw[:], ones_f[:, 0:8], AF.Ln)
        wps = proj_psum.tile([128, 512], F32, tag="s", name="warm")
        for i in range(16):
            nc.tensor.matmul(wps[:], wrm[:, 0:128], wrm[:],
                             start=(i == 0), stop=(i == 15))

        def proj_pieces(tcx):
            """Projection of chunk tcx as a list of thunks (matmul groups).

            The thunks are interleaved into the previous chunk's attention
            stream: each is a dense 8-matmul accumulation group (or similar)
            with no entry semaphore wait, which keeps the PE array busy
            enough that the HAM clock gate stays at 2.4 GHz, and hides the
            whole projection phase inside attention's Scalar-bound slack."""
            ts = slice(tcx * 512, (tcx + 1) * 512)
            qb = qkpool.tile([128, 4, 512], F32, tag="qb", name=f"qb{tcx}")
            sq = sqpool.tile([128, 4, 512], BF16, tag="sq", name=f"sq{tcx}")
            st = {}

            def v_mm(tt, half):
                t = tcx * 4 + tt
                if half == 0:
                    st[f"pv{tt}"] = proj_psum.tile([128, HD], F32, tag="s",
                                                   name=f"pv{t}")
                pv = st[f"pv{tt}"]
                for dk in range(4 * half, 4 * half + 4):
                    nc.tensor.matmul(pv[:],
                                     xts[:, tcx, dk, tt * 128:(tt + 1) * 128],
                                     wvt[:, dk, :],
                                     start=(dk == 0), stop=(bz and dk == NKD - 1))
                if half == 1 and not bz:
                    nc.tensor.matmul(pv[:], ones_b[:], bvr[:],
                                     start=False, stop=True)

            def v_post(tt):
                t = tcx * 4 + tt
                nc.vector.tensor_copy(
                    v_ext[:, t, :, 0:D],
                    st[f"pv{tt}"][:].rearrange("p (h d) -> p h d", h=HPC))

            def pj_mm(j, half):
                if half == 0:
                    st[f"pj{j}"] = proj_psum.tile([128, 512], F32, tag="s",
                                                  name=f"pj{tcx}_{j}")
                pj = st[f"pj{j}"]
                for dk in range(4 * half, 4 * half + 4):
                    nc.tensor.matmul(pj[:],
                                     wqkt[:, dk, j * 128:(j + 1) * 128],
                                     xts[:, tcx, dk, :],
                                     start=(dk == 0), stop=(dk == NKD - 1))

            def pj_post(j):
                pj = st[f"pj{j}"]
                # keep the PSUM evacuation OFF the Scalar queue: Scalar is
                # the exp-stream pacing engine and these copies land inside
                # Scalar-critical attention windows (Identity only when a
                # bias must be applied; Identity is in every ACT table set,
                # Square is NOT — it would force ~2.7us table reloads)
                if bz:
                    nc.vector.tensor_copy(qb[:, j, :], pj[:])
                else:
                    nc.scalar.activation(qb[:, j, :], pj[:], AF.Identity,
                                         bias=bqk[:, j:j + 1])
                nc.vector.tensor_mul(sq[:, j, :], qb[:, j, :], qb[:, j, :])

            def ns_mm():
                ns = spool.tile([8, 512], F32, tag="s", name=f"ns{tcx}")
                for j in range(4):
                    nc.tensor.matmul(ns[:], oblk[:, j, :], sq[:, j, :],
                                     start=(j == 0), stop=(j == 3))
                st["ns"] = ns

            def ns_post():
                # rsqrt = exp(-0.5*ln): short chain on Scalar; the Exp
                # writes f32r directly for the broadcast matmuls
                sn = gpool.tile([8, 512], F32, tag="g", name=f"sn{tcx}")
                nc.scalar.activation(sn[:], st["ns"][:], AF.Ln)
                rr = gpool.tile([8, 512], F32R, tag="g", name=f"rr{tcx}")
                nc.scalar.activation(rr[:], sn[:], AF.Exp, scale=-0.5)
                st["rr"] = rr

            def bc_mm(j):
                bcp = spool.tile([128, 512], F32, tag="s",
                                 name=f"bcp{tcx}_{j}")
                nc.tensor.matmul(bcp[:], tblk_r[:, j * 128:(j + 1) * 128],
                                 st["rr"][:], start=True, stop=True)
                st[f"bcp{j}"] = bcp

            def bc_post(j):
                dst = qhatT if j < 2 else khatT
                nc.vector.tensor_mul(dst[:, j % 2, ts], qb[:, j, :],
                                     st[f"bcp{j}"][:])

            # (pe_thunk, post_thunk, cost_ns); the post runs one filler
            # step later so cross-queue emission order stays deadlock-free
            # norm/bcast chain BEFORE the V tiles: the chain's DVE
            # multiplies (which gate the next chunk's scores via qhat/khat)
            # must not queue behind ~3.5us of V-tile casts on Vector
            pieces = []
            for j in range(4):
                pieces.append((lambda j=j: pj_mm(j, 0), None, 870))
                pieces.append((lambda j=j: pj_mm(j, 1),
                               lambda j=j: pj_post(j), 870))
            pieces.append((ns_mm, ns_post, 870))
            for j in range(4):
                pieces.append((lambda j=j: bc_mm(j),
                               lambda j=j: bc_post(j), 430))
            for t in range(4):
                pieces.append((lambda t=t: v_mm(t, 0), None, 440))
                pieces.append((lambda t=t: v_mm(t, 1),
                                lambda t=t: v_post(t), 440))
            return pieces

        for pe_fn, post_fn, _cost in proj_pieces(0):
            pe_fn()
            if post_fn is not None:
                post_fn()

        # =====================================================================
        # Attention: one GLOBAL software pipeline across all q-chunks.
        # Iteration g emits: score-pair(g) [1 matmul pair into a 2-bank
        # tile], stitches scheduled for g (softmax tails of finished pairs),
        # exp(g-1) [ONE activation for both heads], AV(g-2), then a paced
        # quantum of filler (next chunk's projection / prev chunk's output
        # projection). Chunk tails therefore ride inside the next chunk's
        # iterations and the PE never drains at a boundary.
        # =====================================================================
        pend_exp = None
        pend_av = None
        filler = []
        ofiller = []
        pend_posts = []
        stitches = {}
        g_iter = 0
        fill_end = 0
        g_total = (sum(2 * (4 * q + 4) for q in range(NQC))
                   if variant == "causal" else 2 * NT * NQC)

        def run_piece(pe_fn, post_fn):
            nonlocal pend_posts
            for f in pend_posts:
                f()
            pend_posts = []
            pe_fn()
            if post_fn is not None:
                pend_posts.append(post_fn)

        def drain_filler(o=False):
            nonlocal pend_posts
            while filler:
                pe_fn, post_fn, _c = filler.pop(0)
                run_piece(pe_fn, post_fn)
            while o and ofiller:
                pe_fn, post_fn, _c = ofiller.pop(0)
                run_piece(pe_fn, post_fn)
            for f in pend_posts:
                f()
            pend_posts = []

        def pop_filler():
            # next chunk's projections pace toward the chunk boundary;
            # output projections (no deadline) pace toward the global end
            if filler:
                left = max(1, fill_end - 2 - g_iter)
                budget = sum(c for _, _, c in filler) / left
                spent = 0
                while filler and spent < budget:
                    pe_fn, post_fn, c = filler.pop(0)
                    run_piece(pe_fn, post_fn)
                    spent += c
            if ofiller:
                left = max(1, g_total - g_iter)
                budget = sum(c for _, _, c in ofiller) / left
                spent = 0
                while ofiller and spent < budget:
                    pe_fn, post_fn, c = ofiller.pop(0)
                    run_piece(pe_fn, post_fn)
                    spent += c

        def at_iter(g, fn):
            stitches.setdefault(g, []).append(fn)

        def do_exps(pe):
            """ONE exp activation covers both heads of the score pair"""
            eqc, hp, kb, sp, uA, uB, off, ai, mk = pe
            if mk is not None:
                for hm in range(2):
                    nc.vector.tensor_add(sp[:, hm, :], sp[:, hm, :], mk[:])
            es = espool.tile([128, 2, 512], BF16, tag="es")
            nc.scalar.activation(es[:, :, 0:512 - off], sp[:, :, off:512],
                                 AF.Exp)
            if ai is not None:
                # staircase nontrivial only in the first 128 live cols
                for hm in range(2):
                    nc.vector.tensor_mul(
                        es[:, hm, 0:128], es[:, hm, 0:128],
                        pat_sb[:, ai, off:off + 128])
            return pe[:3] + (es,) + pe[4:]

        def do_avs(pa):
            eqc, hp, kb, es, uA, uB, off, ai, mk = pa
            nkb_l = 4 * eqc + 4 if variant == "causal" else NT
            for hm, u_h in ((0, uA), (1, uB)):
                nc.tensor.matmul(u_h[:, off:512],
                                 v_ext[:, kb, 2 * hp + hm, :],
                                 es[:, hm, 0:512 - off],
                                 start=(kb == 0), stop=(kb == nkb_l - 1),
                                 skip_group_check=True)

        def rotate(new_pe):
            nonlocal pend_exp, pend_av
            av_next = do_exps(pend_exp) if pend_exp is not None else None
            if pend_av is not None:
                do_avs(pend_av)
            pend_av = av_next
            pend_exp = new_pe

        def flush_tail():
            nonlocal pend_exp, pend_av
            av_next = do_exps(pend_exp) if pend_exp is not None else None
            if pend_av is not None:
                do_avs(pend_av)
            if av_next is not None:
                do_avs(av_next)
            pend_exp = pend_av = None

        # Per-pair softmax tail: numerators+denominators leave PSUM as soon
        # as the pair's last AV lands (freeing the u banks for the next
        # pair); ONE approx-reciprocal + broadcast + multiply per head.
        def save_pair(sqc, p, uA, uB):
            # denominators first: the next stitch (reciprocal) needs them
            gs = [gpool.tile([1, 512], F32, tag="g",
                             name=f"g{sqc}_{p}_{hm}") for hm in range(2)]
            nc.vector.tensor_copy(gs[0][:], uA[D:D + 1, :])
            nc.vector.tensor_copy(gs[1][:], uB[D:D + 1, :])
            ua = uapool.tile([128, 512], F32, tag="ua", name=f"ua{sqc}_{p}")
            nc.vector.tensor_copy(ua[0:64, :], uA[0:D, :])
            nc.vector.tensor_copy(ua[64:128, :], uB[0:D, :])
            return ua, gs

        def recip_pair(sqc, p, gs):
            # single-op DVE approx reciprocal (~18 bits), written as f32r
            # so the broadcast matmul can consume it directly
            from concourse.dve_ops import (
                RECIP_APPROX_FAST_CONSTS, RECIPROCAL_APPROX_FAST)
            c = RECIP_APPROX_FAST_CONSTS
            rdg = []
            for hm in range(2):
                r = gpool.tile([1, 512], BF16, tag="g",
                               name=f"rdg{sqc}_{p}_{hm}")
                nc.vector._custom_dve(
                    RECIPROCAL_APPROX_FAST, out=r[:], in0=gs[hm][:],
                    s0=c["s0"], s1=c["s1"], imm2=c["imm2"])
                rdg.append(r)
            return rdg

        def gnorm_pair(sqc, p, ua, rdg):
            for hm in range(2):
                bcd = spool.tile([64, 512], F32, tag="s",
                                 name=f"gbc{sqc}_{p}_{hm}")
                nc.tensor.matmul(bcd[:], ones64_b[:], rdg[hm][:],
                                 start=True, stop=True)
                nc.vector.tensor_mul(
                    aT[hm * 64:(hm + 1) * 64, p,
                       sqc * 512:(sqc + 1) * 512],
                    ua[hm * 64:(hm + 1) * 64, :], bcd[:])

        ysbs = {}

        def oproj_jt(oqc, jt):
            """output projection for chunk oqc, one jt row-block"""
            if jt == 0:
                ysbs[oqc] = ypool.tile([128, NKD, 512], BF16, tag="ys",
                                       name=f"ys{oqc}")
            ysb = ysbs[oqc]
            yp = proj_psum.tile([128, 512], F32, tag="s",
                                name=f"y{oqc}_{jt}")
            for kc in range(2):
                nc.tensor.matmul(
                    yp[:], wot[:, kc, jt * 128:(jt + 1) * 128],
                    aT[:, kc, oqc * 512:(oqc + 1) * 512],
                    start=(kc == 0), stop=(kc == 1))
            if bz:
                nc.vector.tensor_copy(ysb[:, jt, :], yp[:])
            else:
                nc.scalar.activation(ysb[:, jt, :], yp[:], AF.Identity,
                                     bias=bo_sb[:, jt:jt + 1])
            if jt % 2 == 1:
                jl = slice(jt - 1, jt + 1)
                cols = slice((jt - 1) * 512, (jt + 1) * 512)
                nc.sync.dma_start(
                    yt_p.ap()[oqc * 128:(oqc + 1) * 128, cols]
                    .rearrange("p (a f) -> p a f", f=512),
                    ysbs[oqc][:, jl, :])

        def schedule_pair_tail(sqc, p, uA, uB, g_end):
            """save/recip/gnorm ride inside the following iterations"""
            ctx = {}

            def s1():
                ctx["ua"], ctx["gs"] = save_pair(sqc, p, uA, uB)

            def s2():
                ctx["rdg"] = recip_pair(sqc, p, ctx["gs"])

            def s3():
                gnorm_pair(sqc, p, ctx["ua"], ctx["rdg"])
                if p == 1:
                    # output projection of this chunk becomes low-priority
                    # filler now that both halves of aT[:, :, chunk] are
                    # final — paced into the late, Scalar-bound chunks
                    ofiller.extend(
                        (lambda o=sqc, jt=jt: oproj_jt(o, jt), None, 550)
                        for jt in range(NKD))

            at_iter(g_end + 3, s1)
            at_iter(g_end + 4, s2)
            at_iter(g_end + 5, s3)

        # main loop
        for qc in range(NQC):
            kbs = list(range(4 * qc + 4)) if variant == "causal" else list(range(NT))
            nkb = len(kbs)
            mks = {}
            if variant == "mask":
                for kb in kbs:
                    mk = mpool.tile([128, 512], F32, tag="mk",
                                    name=f"mk{qc}_{kb}")
                    nc.sync.dma_start(
                        mk[:], mt_p.ap()[kb * 128:(kb + 1) * 128,
                                         qc * 512:(qc + 1) * 512])
                    mks[kb] = mk

            if qc + 1 < NQC:
                filler.extend(proj_pieces(qc + 1))
            fill_end = g_iter + 2 * nkb

            def score_pair(hp, kb):
                """both heads of pair hp score into one 2-bank tile"""
                off = 0
                if variant == "causal":
                    ai = kb - 4 * qc
                    off = ai * 128 if ai > 0 else 0
                sp = spool.tile([128, 2, 512], F32, tag="s",
                                name=f"s{qc}_{kb}_{hp}")
                for hm in range(2):
                    ho = hm * 64
                    nc.tensor.matmul(
                        sp[:, hm, off:512],
                        khatT[ho:ho + 64, hp, kb * 128:(kb + 1) * 128],
                        qhatT[ho:ho + 64, hp, qc * 512 + off:(qc + 1) * 512],
                        start=True, stop=True)
                return sp, off

            for hp in range(2):
                uA = upsum.tile([D + 1, 512], F32, tag="u",
                                name=f"u{qc}_{2 * hp}")
                uB = upsum.tile([D + 1, 512], F32, tag="u",
                                name=f"u{qc}_{2 * hp + 1}")
                for i, kb in enumerate(kbs):
                    sp, off = score_pair(hp, kb)
                    for fn in stitches.pop(g_iter, []):
                        fn()
                    ai = (kb - 4 * qc if variant == "causal"
                          and kb >= 4 * qc else None)
                    rotate((qc, hp, kb, sp, uA, uB, off, ai,
                            mks.get(kb)))
                    pop_filler()
                    g_iter += 1
                schedule_pair_tail(qc, hp, uA, uB, g_iter - 1)

            if qc + 1 < NQC:
                # next chunk's projections must be fully emitted before its
                # first score reads qhatT/khatT
                drain_filler()

        # tail: flush the pipeline, run remaining stitches, final oproj
        flush_tail()
        for g in sorted(stitches):
            for fn in stitches[g]:
                fn()
        stitches.clear()
        drain_filler(o=True)

    nc.compile()
    return nc


_PROGRAM_CACHE: dict = {}


def _get_program(variant: str, bz: bool = True) -> bass.Bass:
    key = (variant, bz)
    if key not in _PROGRAM_CACHE:
        _PROGRAM_CACHE[key] = build_program(variant, bz)
    return _PROGRAM_CACHE[key]


def _detect_variant(mask: np.ndarray) -> str:
    m = np.asarray(mask).reshape(T, T)
    if not m.any():
        return "zeros"
    tri = np.tril(np.ones((T, T), dtype=bool))
    if np.all(m[tri] == 0.0) and np.all(m[~tri] <= -1e8):
        return "causal"
    return "mask"


def _staircase_patterns() -> np.ndarray:
    kk = np.arange(128)[:, None, None]
    ai = np.arange(4)[None, :, None]
    qq = np.arange(512)[None, None, :]
    return (kk + ai * 128 <= qq).astype(BF)


def build_core_inputs(variant, x, mask, Wq, bq, Wk, bk, Wv, bv, Wo, bo, tau):
    """Host-side shard + pre-transpose + relayout + bf16 cast."""
    x = np.asarray(x, dtype=np.float32)
    Wq = np.asarray(Wq, dtype=np.float32)
    Wk = np.asarray(Wk, dtype=np.float32)
    Wv = np.asarray(Wv, dtype=np.float32)
    Wo = np.asarray(Wo, dtype=np.float32)
    bq = np.asarray(bq, dtype=np.float32)
    bk = np.asarray(bk, dtype=np.float32)
    bv = np.asarray(bv, dtype=np.float32)
    bo = np.asarray(bo, dtype=np.float32)
    tau = np.asarray(tau, dtype=np.float32).reshape(H)

    pat = _staircase_patterns() if variant == "causal" else None
    maskt = (np.ascontiguousarray(
        np.asarray(mask, dtype=np.float32).reshape(T, T).T)
        if variant == "mask" else None)

    oblk = np.zeros((128, 4, 8), dtype=BF)
    for j in range(4):
        oblk[0:64, j, 2 * j] = 1
        oblk[64:128, j, 2 * j + 1] = 1

    def dk_major(a):
        """[NKD*128, F] -> [128, NKD*F] with dk the slower free index"""
        kd, f = a.shape
        return np.ascontiguousarray(
            a.reshape(NKD, 128, f).transpose(1, 0, 2).reshape(128, NKD * f))

    in_maps = []
    for c in range(NCORES):
        b = c // 4
        h0 = (c % 4) * HPC
        sl = slice(h0 * D, (h0 + HPC) * D)
        tblk = np.zeros((8, 512), dtype=np.float32)
        for j in range(4):
            v0 = tau[h0 + 2 * j] if j < 2 else 1.0
            v1 = tau[h0 + 2 * j + 1] if j < 2 else 1.0
            tblk[2 * j, j * 128:j * 128 + 64] = v0
            tblk[2 * j + 1, j * 128 + 64:(j + 1) * 128] = v1
        bqk = np.stack([bq[sl][0:128], bq[sl][128:256],
                        bk[sl][0:128], bk[sl][128:256]], axis=1)
        xT = np.ascontiguousarray(x[b].T)  # [DM, T]
        # chunk-major [qc*128+p, dk*512+t'] = xT[dk*128+p, qc*512+t']
        xtp = np.ascontiguousarray(
            xT.reshape(NKD, 128, NQC, 512).transpose(2, 1, 0, 3)
            .reshape(NQC * 128, NKD * 512))
        wotT = np.ascontiguousarray(Wo[:, sl].T)  # [HD, DM]
        m = {
            "xt": xtp.astype(BF),
            "wqkt": dk_major(np.concatenate(
                [Wq[sl].T, Wk[sl].T], axis=1)).astype(BF),
            "wvt": dk_major(np.ascontiguousarray(Wv[sl].T)).astype(BF),
            "wot": np.ascontiguousarray(
                wotT.reshape(2, 128, DM).transpose(1, 0, 2)
                .reshape(128, 2 * DM)).astype(BF),
            "bqk": np.ascontiguousarray(bqk),
            "bvr": bv[sl].reshape(1, HD).astype(BF),
            "bo": (bo.reshape(NKD, 128).T.copy() if c % 4 == 0
                   else np.zeros((128, NKD), dtype=np.float32)),
            "tblk": tblk,
            "oblk": oblk,
        }
        if variant == "causal":
            m["pat"] = pat
        if variant == "mask":
            m["maskt"] = maskt
        in_maps.append(m)
    return in_maps


def kernel(x, mask, Wq, bq, Wk, bk, Wv, bv, Wo, bo, tau):
    variant = _detect_variant(np.asarray(mask, dtype=np.float32))
    bz = not (np.asarray(bq).any() or np.asarray(bk).any()
              or np.asarray(bv).any() or np.asarray(bo).any())
    nc = _get_program(variant, bz)
    in_maps = build_core_inputs(variant, x, mask, Wq, bq, Wk, bk,
                                Wv, bv, Wo, bo, tau)
    res = run_bass_kernel_spmd(nc, in_maps, list(range(NCORES)))
    out = np.empty((B, T, DM), dtype=np.float32)
    for b in range(B):
        acc = res.results[4 * b]["yt"].astype(np.float32)
        for c in range(4 * b + 1, 4 * b + 4):
            acc += res.results[c]["yt"].astype(np.float32)
        # [qc*128+p, jt*512+t'] -> [DM, T] -> [T, DM]
        y = acc.reshape(NQC, 128, NKD, 512).transpose(2, 1, 0, 3) \
               .reshape(DM, T)
        out[b] = y.T
    return out
